# revision 51
# baseline (speedup 1.0000x reference)
"""ArcTransformer Trainium2 kernel: 8-core SPMD, feature-major layout.

Sharding: core c owns pair (b=c//4, p=c%4) = grid columns be=2c, 2c+1.
Grid/pair stages fully local; one AllGather of pf feeds the task stage
(computed for both batches on every core; host takes core 0's copy).

Layout convention: activations are stored feature-major ("X^T"):
SBUF tile [128, 2, n] = 256 features (2 partition tiles) x n tokens.
All linear layers run with the weight as the stationary operand
(lhsT = W[d_in, d_out]); V projections run token-major (lhsT = X^T
slice) so the attention AV matmul can contract over keys.
"""

import sys
import time

sys.path.insert(0, "/opt/trn_rl_repo")

import numpy as np
import ml_dtypes

BF16 = ml_dtypes.bfloat16

# model constants (fixed by the problem)
B, TWO_P, Hh, Ww, IN_DIM = 2, 8, 26, 26, 11
D, FF, OUT_DIM, NH = 256, 1024, 11, 8
NPF, NTF, NIT, RDIM = 64, 128, 2, 155
DH = D // NH
PP = TWO_P // 2          # 4 pairs per batch
PTR = PP - 1
L = Hh * Ww              # 676
NG = 2 * L               # 1352 grid tokens per core
NEXT = NG + NTF          # 1480 pair-ext tokens
SCALE = 1.0 / np.sqrt(DH)
N_CORES = 8
EPS = 1e-5

_CACHE = {}


# ---------------------------------------------------------------- helpers
def _chunks(n, base=0, step=512):
    out = []
    c = 0
    while c < n:
        cl = min(step, n - c)
        out.append((base + c, cl))
        c += cl
    return out


def _tok_tiles(base, n, slot0):
    """[(token_offset, count, vs_slot)] in 128-token tiles."""
    out = []
    c = 0
    s = slot0
    while c < n:
        cl = min(128, n - c)
        out.append((base + c, cl, s))
        c += cl
        s += 1
    return out


# ---------------------------------------------------------------- builder
def _patch_act_tables():
    """Pin every ACT func to natural_log_exp_and_others (covers Exp/Ln/Copy/
    Relu/Square) so the greedy table assignment emits one LoadActFuncSet
    instead of thrashing between exp_and_others and natural_log (~1.3us per
    reload). Indices into act_info.json are preserved; unrelated sets are
    emptied so they can never match."""
    from concourse import bacc, hw_specs
    if getattr(bacc, "_act_tables_pinned", False):
        return
    orig = hw_specs.get_activation_tables

    def pinned(arch):
        tabs = orig(arch)
        keep = "natural_log_exp_and_others"
        if keep not in tabs:
            return tabs
        return {k: (v if k == keep else set()) for k, v in tabs.items()}

    bacc.get_activation_tables = pinned
    bacc._act_tables_pinned = True


def _build():
    import concourse.bass as bass
    import concourse.mybir as mybir
    import concourse.tile as tile
    from concourse import bacc


    dt = mybir.dt
    AF = mybir.ActivationFunctionType
    OP = mybir.AluOpType
    BF = dt.bfloat16
    F32 = dt.float32

    nc = bacc.Bacc("TRN2", target_bir_lowering=False, debug=False,
                   num_devices=N_CORES)

    def din(name, shape, d=BF):
        return nc.dram_tensor(name, list(shape), d, kind="ExternalInput").ap()

    def dovt(name, shape, d=F32):
        return nc.dram_tensor(name, list(shape), d, kind="ExternalOutput").ap()

    # -------- dram inputs
    xg_d = din("xg", [IN_DIM, NG])
    relx_d = din("relx", [D, NG])
    emb_d = din("emb", [D, L])
    w_in_d = din("w_in", [IN_DIM, D])
    w_out_d = din("w_out", [D, OUT_DIM])
    pf0_d = din("pf0", [D, NPF])
    tf0_d = din("tf0", [D, NTF])
    text0_d = din("text0", [D, NTF])
    io_emb_d = din("io_emb", [D, 2], F32)
    pair_emb_d = din("pair_emb4", [D, PP], F32)
    wd_d = {}
    for ln in ("g", "p", "t"):
        for wn in ("wq", "wk", "wv", "wo", "eq", "ek", "ev", "eo"):
            wd_d[f"{ln}_{wn}"] = din(f"{ln}_{wn}", [D, D])
        wd_d[f"{ln}_w1"] = din(f"{ln}_w1", [D, FF])
        wd_d[f"{ln}_w2"] = din(f"{ln}_w2", [FF, D])
    wd_d["g_wr"] = din("g_wr", [D, D])

    og_d = dovt("og", [OUT_DIM, NG])
    opf_d = dovt("opf", [D, NPF])
    otf_d = dovt("otf", [D, 2 * NTF])

    import os
    KDBG = bool(os.environ.get("KDBG"))
    dbg_d = {}
    if KDBG:
        for nm in ("dbg_gx", "dbg_q", "dbg_kp", "dbg_vs", "dbg_e", "dbg_av",
                   "dbg_dn", "dbg_rr", "dbg_at", "dbg_y", "dbg_z"):
            dbg_d[nm] = dovt(nm, [D, 64])

    from contextlib import ExitStack
    with tile.TileContext(nc, trace_sim=False) as tc, ExitStack() as stk:
        wp = stk.enter_context(tc.tile_pool(name="wp", bufs=1))
        ap_ = stk.enter_context(tc.tile_pool(name="act", bufs=1))
        sp = stk.enter_context(tc.tile_pool(name="scr", bufs=2))
        ep = stk.enter_context(tc.tile_pool(name="expp", bufs=10))
        rp = stk.enter_context(tc.tile_pool(name="rows", bufs=2))
        pp_ = stk.enter_context(tc.tile_pool(name="ps", bufs=2, space="PSUM"))
        dp = stk.enter_context(tc.tile_pool(name="drp", bufs=1, space="DRAM"))

        # -------- load weights to SBUF  [128, K/128, M]
        # round-robin the DMA-triggering engine so the ~30 weight loads fan
        # out over four DGE queues instead of serializing on SP
        dma_engs = [nc.sync]
        wt = {}
        for wi_, (name, apd) in enumerate(wd_d.items()):
            K, M = apd.shape
            kt = max(1, K // 128)
            t = wp.tile([128, kt, M], BF, tag=f"w_{name}", name=f"w_{name}")
            dma_engs[0].dma_start(t[:],
                                        apd.rearrange("(kt p) m -> p kt m", p=128))
            wt[name] = t
        w_in_t = wp.tile([128, 1, D], BF, tag="w_w_in", name="w_w_in")
        nc.sync.dma_start(w_in_t[:IN_DIM, 0], w_in_d[:])
        w_out_t = wp.tile([128, 2, OUT_DIM], BF, tag="w_w_out", name="w_w_out")
        nc.sync.dma_start(w_out_t[:], w_out_d.rearrange("(kt p) m -> p kt m", p=128))

        ones_t = wp.tile([128, 128], BF, tag="ones_t", name="ones_t")
        nc.vector.memset(ones_t[:], 1.0)
        ones_sc = wp.tile([128, 1], BF, tag="ones_sc", name="ones_sc")
        nc.vector.memset(ones_sc[:], 1.0 / D)
        eps_t = wp.tile([128, 1], F32, tag="eps_t", name="eps_t")
        nc.vector.memset(eps_t[:], EPS)

        io_t = wp.tile([128, 2, 2], F32, tag="io_t", name="io_t")
        nc.sync.dma_start(io_t[:], io_emb_d.rearrange("(f p) n -> p f n", p=128))
        pe4_t = wp.tile([128, 2, PP], F32, tag="pe4_t", name="pe4_t")
        nc.sync.dma_start(pe4_t[:], pair_emb_d.rearrange("(f p) n -> p f n", p=128))

        dbg_t = {}
        for nm in dbg_d:
            dbg_t[nm] = wp.tile([128, 2, 64], F32, tag=nm, name=nm)

        def snap_fm(nm, t):
            if KDBG:
                for f in range(2):
                    nc.vector.tensor_copy(dbg_t[nm][:, f, :], t[:, f, 0:64])

        # -------- generic ops
        def proj(w_t, x_t, kparts, msizes, cols, evac, tag="s"):
            """out^T[m,:] = sum_k w[k,m] * x^T[k,:]; evac(ps_ap, mi, c0, cl)."""
            for mi, (moff, mcnt) in enumerate(msizes):
                for (c0, cl) in cols:
                    ps = pp_.tile([128, 1024], F32, tag=tag, name=f"ps_{tag}")
                    nk = len(kparts)
                    for ki, kc in enumerate(kparts):
                        nc.tensor.matmul(
                            ps[:mcnt, :cl],
                            lhsT=w_t[:kc, ki, moff:moff + mcnt],
                            rhs=x_t[:kc, ki, c0:c0 + cl],
                            start=(ki == 0), stop=(ki == nk - 1))
                    evac(ps, mi, c0, cl)

        def vproj(wv_t, x_t, toks, vs_t, dual=False):
            """token-major V: vs[t, kt, :] = x^T[:,t].T @ wv.
            dual: also write a copy at partitions 64.. (for dual-row attn)."""
            for (toff, tcnt, slot) in toks:
                ps = pp_.tile([128, 1024], F32, tag="s", name="ps_v")
                ps2 = pp_.tile([128, 1024], F32, tag="s", name="ps_v2") \
                    if dual else None
                for f in range(2):
                    nc.tensor.matmul(
                        ps[:tcnt, :D],
                        lhsT=x_t[:, f, toff:toff + tcnt],
                        rhs=wv_t[:, f, :],
                        start=(f == 0), stop=(f == 1))
                    if dual:
                        nc.tensor.matmul(
                            ps2[64:64 + tcnt, :D],
                            lhsT=x_t[:, f, toff:toff + tcnt],
                            rhs=wv_t[:, f, :],
                            start=(f == 0), stop=(f == 1),
                            tile_position=(0, 64))
                nc.vector.tensor_copy(vs_t[:tcnt, slot, :], ps[:tcnt, :D])
                if dual:
                    nc.vector.tensor_copy(vs_t[64:64 + tcnt, slot, :],
                                          ps2[64:64 + tcnt, :D])

        def attn_core(q_t, q_off, q_len, kp_t, vs_t, toks, out_t, dbg=False,
                      dual=False):
            """softmax(Q_h.K'_h^T).V_h for 8 heads -> out_t[:, :, q_off:+q_len].
            dual: kv <= 64 tokens; stack head pairs on partitions 0-63/64-127
            (requires vs duplicated at rows 64..)."""
            qch = _chunks(q_len)
            nt = len(toks)
            if dual:
                assert nt == 1 and toks[0][1] <= 64
                toff, tcnt, slot = toks[0]
                for f in range(2):
                    av = pp_.tile([128, 1024], F32, tag="acc", name="ps_av")
                    dn = pp_.tile([128, 1024], F32, tag="acc", name="ps_dn")
                    nc.vector.memset(av[:, :q_len], 0.0)
                    nc.vector.memset(dn[:, :q_len], 0.0)
                    for pr in range(2):
                        hb0, hb1 = 64 * pr, 64 * pr + 32
                        sps = pp_.tile([128, 1024], F32, tag="s", name="ps_sc")
                        for (c0, cl) in qch:
                            nc.tensor.matmul(
                                sps[:tcnt, c0:c0 + cl],
                                lhsT=kp_t[hb0:hb0 + 32, f, toff:toff + tcnt],
                                rhs=q_t[hb0:hb0 + 32, f,
                                        q_off + c0:q_off + c0 + cl],
                                start=True, stop=True, tile_position=(hb0, 0))
                            nc.tensor.matmul(
                                sps[64:64 + tcnt, c0:c0 + cl],
                                lhsT=kp_t[hb1:hb1 + 32, f, toff:toff + tcnt],
                                rhs=q_t[hb1:hb1 + 32, f,
                                        q_off + c0:q_off + c0 + cl],
                                start=True, stop=True, tile_position=(hb1, 64),
                                skip_group_check=True)
                        e2t = ep.tile([128, 676], BF, tag="expS", name="expS")
                        nc.scalar.activation(e2t[:, :q_len], sps[:, :q_len],
                                             AF.Exp)
                        for half, hb in ((0, hb0), (1, hb1)):
                            rbase = 64 * half
                            for (c0, cl) in qch:
                                nc.tensor.matmul(
                                    av[hb:hb + 32, c0:c0 + cl],
                                    lhsT=vs_t[rbase:rbase + tcnt, slot,
                                              f * 128 + hb:f * 128 + hb + 32],
                                    rhs=e2t[rbase:rbase + tcnt, c0:c0 + cl],
                                    start=False, stop=False,
                                    tile_position=(rbase, hb),
                                    skip_group_check=True)
                                nc.tensor.matmul(
                                    dn[hb:hb + 1, c0:c0 + cl],
                                    lhsT=ones_t[rbase:rbase + tcnt, :1],
                                    rhs=e2t[rbase:rbase + tcnt, c0:c0 + cl],
                                    start=False, stop=False,
                                    tile_position=(rbase, hb),
                                    skip_group_check=True)
                    rr = sp.tile([128, 676], BF, tag="rr", name="rr")
                    with nc.allow_low_precision(reason="softmax recip bf16"):
                        nc.vector.reciprocal(rr[:97, :q_len], dn[:97, :q_len])
                    rb = pp_.tile([128, 1024], F32, tag="acc", name="ps_rb")
                    for hh in range(4):
                        hb = 32 * hh
                        for (c0, cl) in qch:
                            nc.tensor.matmul(rb[hb:hb + 32, c0:c0 + cl],
                                             lhsT=ones_t[hb:hb + 1, :32],
                                             rhs=rr[hb:hb + 1, c0:c0 + cl],
                                             start=True, stop=True,
                                             tile_position=(hb, hb))
                    rb_sb = sp.tile([128, 676], BF, tag="rb", name="rb_sb")
                    nc.vector.tensor_copy(rb_sb[:, :q_len], rb[:, :q_len])
                    nc.vector.tensor_mul(out_t[:, f, q_off:q_off + q_len],
                                         av[:, :q_len], rb_sb[:, :q_len])
                return
            for f in range(2):
                av = pp_.tile([128, 1024], F32, tag="acc", name="ps_av")
                dn = pp_.tile([128, 1024], F32, tag="acc", name="ps_dn")
                nc.vector.memset(av[:, :q_len], 0.0)
                nc.vector.memset(dn[:, :q_len], 0.0)
                for kt, (toff, tcnt, slot) in enumerate(toks):
                    es = []
                    if False and q_len <= 128:
                        # pack the 4 heads' scores side by side -> one exp call
                        sps = pp_.tile([128, 1024], F32, tag="s", name="ps_sc")
                        for hh in range(4):
                            hb = 32 * hh
                            nc.tensor.matmul(
                                sps[:tcnt, hh * q_len:(hh + 1) * q_len],
                                lhsT=kp_t[hb:hb + 32, f, toff:toff + tcnt],
                                rhs=q_t[hb:hb + 32, f, q_off:q_off + q_len],
                                start=True, stop=True,
                                tile_position=(hb, 0))
                        e4 = ep.tile([128, 676], BF, tag="expS", name="expS")
                        nc.scalar.activation(e4[:tcnt, :4 * q_len],
                                             sps[:tcnt, :4 * q_len], AF.Exp)
                        es = [e4[:, hh * q_len:(hh + 1) * q_len]
                              for hh in range(4)]
                    else:
                        for hh in range(4):
                            hb = 32 * hh
                            sps = pp_.tile([128, 1024], F32, tag="s",
                                           name="ps_sc")
                            for (c0, cl) in qch:
                                nc.tensor.matmul(
                                    sps[:tcnt, c0:c0 + cl],
                                    lhsT=kp_t[hb:hb + 32, f, toff:toff + tcnt],
                                    rhs=q_t[hb:hb + 32, f,
                                            q_off + c0:q_off + c0 + cl],
                                    start=True, stop=True,
                                    tile_position=(hb, 0))
                            e = ep.tile([128, 676], BF, tag="expS", name="expS")
                            nc.scalar.activation(e[:tcnt, :q_len],
                                                 sps[:tcnt, :q_len], AF.Exp)
                            if dbg and f == 0 and kt == 0 and hh == 0:
                                nc.vector.tensor_copy(dbg_t["dbg_e"][:, 0, :],
                                                      e[:128, 0:64])
                            es.append(e)
                    for hh in range(4):
                        hb = 32 * hh
                        for (c0, cl) in qch:
                            nc.tensor.matmul(
                                av[hb:hb + 32, c0:c0 + cl],
                                lhsT=vs_t[:tcnt, slot, f * 128 + hb:f * 128 + hb + 32],
                                rhs=es[hh][:tcnt, c0:c0 + cl],
                                start=False, stop=False,
                                tile_position=(0, hb),
                                skip_group_check=True)
                            nc.tensor.matmul(
                                dn[hb:hb + 1, c0:c0 + cl],
                                lhsT=ones_t[:tcnt, :1],
                                rhs=es[hh][:tcnt, c0:c0 + cl],
                                start=False, stop=False,
                                tile_position=(0, hb),
                                skip_group_check=True)
                rr = sp.tile([128, 676], BF, tag="rr", name="rr")
                with nc.allow_low_precision(reason="softmax recip rows in bf16"):
                    nc.vector.reciprocal(rr[:97, :q_len], dn[:97, :q_len])
                if dbg and f == 0:
                    nc.vector.tensor_copy(dbg_t["dbg_av"][:, 0, :], av[:, 0:64])
                    nc.vector.tensor_copy(dbg_t["dbg_dn"][:, 0, :], dn[:, 0:64])
                    nc.vector.tensor_copy(dbg_t["dbg_rr"][:97, 0, :], rr[:97, 0:64])
                rb = pp_.tile([128, 1024], F32, tag="acc", name="ps_rb")
                for hh in range(4):
                    hb = 32 * hh
                    for (c0, cl) in qch:
                        nc.tensor.matmul(rb[hb:hb + 32, c0:c0 + cl],
                                         lhsT=ones_t[hb:hb + 1, :32],
                                         rhs=rr[hb:hb + 1, c0:c0 + cl],
                                         start=True, stop=True,
                                         tile_position=(hb, hb))
                rb_sb = sp.tile([128, 676], BF, tag="rb", name="rb_sb")
                nc.vector.tensor_copy(rb_sb[:, :q_len], rb[:, :q_len])
                nc.vector.tensor_mul(out_t[:, f, q_off:q_off + q_len],
                                     av[:, :q_len], rb_sb[:, :q_len])

        def layer_norm(y_t, n, z_t):
            """z = (y - mu)/sqrt(var+eps), per token (feature-dim stats)."""
            sq = sp.tile([128, 2, NEXT], BF, tag="sq", name="sq", bufs=1)
            for f in range(2):
                nc.vector.tensor_mul(sq[:, f, :n], y_t[:, f, :n], y_t[:, f, :n])
            mu = rp.tile([1, NEXT], F32, tag="mu", name="mu", bufs=1)
            e2 = rp.tile([1, NEXT], F32, tag="e2", name="e2", bufs=1)
            for (c0, cl) in _chunks(n):
                for src, dst in ((y_t, mu), (sq, e2)):
                    ps = pp_.tile([128, 1024], F32, tag="s", name="ps_st")
                    for f in range(2):
                        nc.tensor.matmul(ps[:1, :cl],
                                         lhsT=ones_sc[:, :1],
                                         rhs=src[:, f, c0:c0 + cl],
                                         start=(f == 0), stop=(f == 1))
                    nc.vector.tensor_copy(dst[:1, c0:c0 + cl], ps[:1, :cl])
            rs1 = rp.tile([1, NEXT], F32, tag="rs1", name="rs1", bufs=1)
            rs2 = rp.tile([1, NEXT], F32, tag="rs2", name="rs2", bufs=1)
            nc.vector.tensor_mul(rs1[:1, :n], mu[:1, :n], mu[:1, :n])    # mu^2
            nc.vector.tensor_sub(rs2[:1, :n], e2[:1, :n], rs1[:1, :n])   # var
            nc.scalar.activation(rs1[:1, :n], rs2[:1, :n], AF.Ln,
                                 bias=eps_t[:1, :1])
            a_r = rs2
            nc.scalar.activation(a_r[:1, :n], rs1[:1, :n], AF.Exp, scale=-0.5)
            c_r = rs1
            nc.vector.scalar_tensor_tensor(c_r[:1, :n], mu[:1, :n], -1.0,
                                           a_r[:1, :n], OP.mult, OP.mult)
            ab = sp.tile([128, NEXT], F32, tag="ab", name="ab", bufs=1)
            cb = sp.tile([128, NEXT], F32, tag="cb", name="cb", bufs=1)
            nc.gpsimd.partition_broadcast(ab[:, :n], a_r[:1, :n])
            nc.gpsimd.partition_broadcast(cb[:, :n], c_r[:1, :n])
            for f in range(2):
                nc.vector.tensor_mul(z_t[:, f, :n], y_t[:, f, :n], ab[:, :n])
                nc.vector.tensor_add(z_t[:, f, :n], z_t[:, f, :n], cb[:, :n])

        D2 = [128, 128]
        M2 = [(0, 128), (128, 128)]

        def xf_layer(pre, x_t, n, win_self, vs_self_n, ext_t, ext_n, win_ext,
                     vs_ext_n, out_t, krel_t=None, dbg=False):
            """one transformer layer; LN3 output written to out_t[:, :, :n]."""
            cols = _chunks(n)
            q_t = ap_.tile([128, 2, NG], BF, tag="q_t", name="q_t")
            kp_t = ap_.tile([128, 2, NEXT], BF, tag="kp_t", name="kp_t")
            at_t = ap_.tile([128, 2, NG], BF, tag="at_t", name="at_t")
            y_t = ap_.tile([128, 2, NG], BF, tag="y_t", name="y_t")
            z1_t = ap_.tile([128, 2, NG], BF, tag="z1_t", name="z1_t")
            z2_t = ap_.tile([128, 2, NG], BF, tag="z2_t", name="z2_t")
            vs_t = ap_.tile([128, 12, D], BF, tag="vs_t", name="vs_t")

            def cp(dst):
                def f(ps, mi, c0, cl):
                    nc.any.tensor_copy(dst[:, mi, c0:c0 + cl], ps[:, :cl])
                return f

            def addk(dst, k_t):
                def f(ps, mi, c0, cl):
                    nc.vector.tensor_add(dst[:, mi, c0:c0 + cl], ps[:, :cl],
                                         k_t[:, mi, c0:c0 + cl])
                return f

            # ---- self attention
            proj(wt[pre + "_wq"], x_t, D2, M2, cols, cp(q_t))
            if krel_t is not None:
                proj(wt[pre + "_wk"], x_t, D2, M2, cols, addk(kp_t, krel_t))
            else:
                proj(wt[pre + "_wk"], x_t, D2, M2, cols, cp(kp_t))
            alltoks = [t for (_, _, tl) in win_self for t in tl]
            vproj(wt[pre + "_wv"], x_t, alltoks, vs_t)
            if dbg:
                snap_fm("dbg_q", q_t)
                snap_fm("dbg_kp", kp_t)
                nc.vector.tensor_copy(dbg_t["dbg_vs"][:, 0, :],
                                      vs_t[:128, 0, 0:64])
                nc.vector.tensor_copy(dbg_t["dbg_vs"][:, 1, :],
                                      vs_t[:128, 0, 64:128])
            for wi, (qo, ql, tl) in enumerate(win_self):
                attn_core(q_t, qo, ql, kp_t, vs_t, tl, at_t,
                          dbg=(dbg and wi == 0))
            if dbg:
                snap_fm("dbg_at", at_t)
            proj(wt[pre + "_wo"], at_t, D2, M2, cols, addk(y_t, x_t))
            layer_norm(y_t, n, z1_t)
            if dbg:
                snap_fm("dbg_y", y_t)
                snap_fm("dbg_z", z1_t)
            # ---- external attention (K2/V2 only depend on ext_t, so they can
            # overlap the self-attention + LN1 work -> separate tiles)
            kp2_t = ap_.tile([128, 2, NEXT], BF, tag="kp2_t", name="kp2_t")
            proj(wt[pre + "_eq"], z1_t, D2, M2, cols, cp(q_t))
            proj(wt[pre + "_ek"], ext_t, D2, M2, _chunks(ext_n), cp(kp2_t))
            exttoks = [t for (_, _, tl) in win_ext for t in tl]
            seen = set()
            uniq = [t for t in exttoks if not (t in seen or seen.add(t))]
            ext_dual = False
            vproj(wt[pre + "_ev"], ext_t, uniq, vs_t, dual=ext_dual)
            for (qo, ql, tl) in win_ext:
                attn_core(q_t, qo, ql, kp2_t, vs_t, tl, at_t, dual=ext_dual)
            proj(wt[pre + "_eo"], at_t, D2, M2, cols, addk(y_t, z1_t))
            layer_norm(y_t, n, z2_t)
            # ---- FFN
            h_t = ap_.tile([128, 8, NG], BF, tag="h_t", name="h_t")

            def relu_evac(ps, mi, c0, cl):
                if mi % 2 == 0:
                    nc.scalar.activation(h_t[:, mi, c0:c0 + cl], ps[:, :cl],
                                         AF.Relu)
                else:
                    nc.vector.tensor_scalar_max(h_t[:, mi, c0:c0 + cl],
                                                ps[:, :cl], 0.0)

            proj(wt[pre + "_w1"], z2_t, D2, [(i * 128, 128) for i in range(8)],
                 cols, relu_evac)
            proj(wt[pre + "_w2"], h_t, [128] * 8, M2, cols, addk(y_t, z2_t))
            layer_norm(y_t, n, out_t)

        # ============================ forward ============================
        # ---- grid embedding
        gx = ap_.tile([128, 2, NEXT], BF, tag="gx", name="gx")
        xg_t = ap_.tile([128, 1, NG], BF, tag="q_t", name="xg_t")
        nc.sync.dma_start(xg_t[:IN_DIM, 0], xg_d[:])
        emb_t = ap_.tile([128, 2, L], BF, tag="kp_t", name="emb_t")
        nc.sync.dma_start(emb_t[:], emb_d.rearrange("(f p) n -> p f n", p=128))
        nc.sync.dma_start(gx[:, :, NG:NEXT],
                          text0_d.rearrange("(f p) n -> p f n", p=128))

        def emb_evac(ps, mi, c0, cl):
            ec = c0 % L
            nc.vector.tensor_add(gx[:, mi, c0:c0 + cl], ps[:, :cl],
                                 emb_t[:, mi, ec:ec + cl])

        gcols = _chunks(L) + _chunks(L, base=L)
        proj(w_in_t, xg_t, [IN_DIM], M2, gcols, emb_evac)
        snap_fm("dbg_gx", gx)

        # ---- cached rel-prior projection (added to K in both grid layers)
        relx_t = ap_.tile([128, 2, NG], BF, tag="at_t", name="relx_t")
        nc.sync.dma_start(relx_t[:], relx_d.rearrange("(f p) n -> p f n", p=128))
        krel_t = ap_.tile([128, 2, NG], BF, tag="krel_t", name="krel_t")

        def krel_evac(ps, mi, c0, cl):
            nc.any.tensor_copy(krel_t[:, mi, c0:c0 + cl], ps[:, :cl])

        proj(wt["g_wr"], relx_t, D2, M2, _chunks(NG), krel_evac)

        # ---- pf0 / tf0
        pf_t = ap_.tile([128, 2, NPF], BF, tag="pf_t", name="pf_t")
        nc.sync.dma_start(pf_t[:], pf0_d.rearrange("(f p) n -> p f n", p=128))

        g_self_win = [(0, L, _tok_tiles(0, L, 0)), (L, L, _tok_tiles(L, L, 6))]
        g_ext_win = [(0, L, [(0, NPF, 0)]), (L, L, [(0, NPF, 0)])]

        # ---- grid layer, iteration 0 (ext kv = pf0)
        xf_layer("g", gx, NG, g_self_win, 12, pf_t, NPF, g_ext_win, 1,
                 gx, krel_t=krel_t, dbg=KDBG)

        # ---- in/out embeddings persist into g
        for f in range(2):
            for io in range(2):
                nc.vector.tensor_scalar_add(gx[:, f, io * L:(io + 1) * L],
                                            gx[:, f, io * L:(io + 1) * L],
                                            io_t[:, f, io:io + 1])

        # ---- pair layer: target pf0 -> pf1; ext = [g4 | text0] = gx
        pf1_t = ap_.tile([128, 2, NPF], BF, tag="pf1_t", name="pf1_t")
        p_self_win = [(0, NPF, [(0, NPF, 0)])]
        p_ext_win = [(0, NPF, _tok_tiles(0, NEXT, 0))]
        xf_layer("p", pf_t, NPF, p_self_win, 1, gx, NEXT, p_ext_win, 12, pf1_t)

        # pf output (f32) + collective input bounce
        opf_sb = ap_.tile([128, 2, NPF], F32, tag="opf_sb", name="opf_sb")
        for f in range(2):
            nc.any.tensor_copy(opf_sb[:, f, :], pf1_t[:, f, :])
        nc.sync.dma_start(opf_d.rearrange("(f p) n -> p f n", p=128), opf_sb[:])
        pf_bnc = dp.tile([D, NPF], BF, tag="pf_bnc", name="pf_bnc")
        nc.gpsimd.dma_start(pf_bnc.rearrange("(f p) n -> p f n", p=128), pf1_t[:])
        pf_gth = dp.tile([N_CORES, D, NPF], BF, tag="pf_gth", name="pf_gth",
                         addr_space="Shared")
        nc.gpsimd.collective_compute(
            "AllGather", mybir.AluOpType.bypass,
            replica_groups=[list(range(N_CORES))],
            ins=[pf_bnc[:].opt()], outs=[pf_gth[:].opt()])

        # ---- grid layer, iteration 1 (ext kv = own pf1 column)
        xf_layer("g", gx, NG, g_self_win, 12, pf1_t, NPF, g_ext_win, 1,
                 gx, krel_t=krel_t)

        # ---- grid output projection
        og_sb = ap_.tile([128, NG], F32, tag="og_sb", name="og_sb")

        def og_evac(ps, mi, c0, cl):
            nc.any.tensor_copy(og_sb[:OUT_DIM, c0:c0 + cl], ps[:OUT_DIM, :cl])

        proj(w_out_t, gx, D2, [(0, OUT_DIM)], _chunks(NG), og_evac)
        nc.sync.dma_start(og_d[:], og_sb[:OUT_DIM, :])

        # ---- task layer inputs: ext2 from gathered pf (both batches)
        ext2_t = ap_.tile([128, 2, 512], BF, tag="ext2_t", name="ext2_t")
        g2 = pf_gth.rearrange("lo (f p) (p4 bb hi) -> f bb p p4 hi lo",
                              p=128, bb=2, hi=8)
        for f in range(2):
            for b in range(2):
                dst = ext2_t[:, f, b * 256:(b + 1) * 256].rearrange(
                    "p (p4 hi lo) -> p p4 hi lo", p4=4, hi=8)
                for lo in range(8):
                    nc.sync.dma_start(dst[:, :, :, lo], g2[f, b, :, :, :, lo])
        for f in range(2):
            for b in range(2):
                for p4 in range(PP):
                    c0 = b * 256 + p4 * 64
                    nc.vector.tensor_scalar_add(ext2_t[:, f, c0:c0 + 64],
                                                ext2_t[:, f, c0:c0 + 64],
                                                pe4_t[:, f, p4:p4 + 1])

        tf_t = ap_.tile([128, 2, 2 * NTF], BF, tag="tf_t", name="tf_t")
        for b in range(2):
            nc.sync.dma_start(tf_t[:, :, b * NTF:(b + 1) * NTF],
                              tf0_d.rearrange("(f p) n -> p f n", p=128))
        tf1_t = ap_.tile([128, 2, 2 * NTF], BF, tag="tf1_t", name="tf1_t")
        t_self_win = [(0, NTF, [(0, NTF, 0)]), (NTF, NTF, [(NTF, NTF, 1)])]
        t_ext_win = [(0, NTF, [(0, 128, 0), (128, 128, 1)]),
                     (NTF, NTF, [(256, 128, 2), (384, 128, 3)])]
        xf_layer("t", tf_t, 2 * NTF, t_self_win, 2, ext2_t, 512, t_ext_win, 4,
                 tf1_t)
        otf_sb = ap_.tile([128, 2, 2 * NTF], F32, tag="otf_sb", name="otf_sb")
        for f in range(2):
            nc.any.tensor_copy(otf_sb[:, f, :], tf1_t[:, f, :])
        nc.sync.dma_start(otf_d.rearrange("(f p) n -> p f n", p=128), otf_sb[:])

        for nm in dbg_d:
            nc.sync.dma_start(dbg_d[nm].rearrange("(f p) n -> p f n", p=128),
                              dbg_t[nm][:])

    import os
    if not os.environ.get("KERNEL_BUILD_ONLY"):
        nc.compile()
    return nc


# ---------------------------------------------------------------- host side
def _np32(x):
    return np.asarray(x, dtype=np.float32)


def _prep_inputs(grids, grid_prior, params):
    p = params
    f32 = _np32

    def bf(x):
        return np.ascontiguousarray(f32(x)).astype(BF16)

    shared = {}
    # grid embedding (pos added twice + seq embeddings + b_in), transposed
    gpos = f32(p["grid_pos"])[:Ww, :Hh, :].reshape(L, D)
    seq = np.where(np.arange(L)[:, None] < PTR * 2,
                   f32(p["train_emb"])[None], f32(p["test_emb"])[None])
    seq[L - 1] += f32(p["test_out_emb"])
    emb = f32(p["b_in"])[None] + 2.0 * gpos + seq
    shared["emb"] = bf(emb.T)
    shared["w_in"] = bf(p["W_in"])
    shared["w_out"] = bf(p["W_out"])
    pf_emb = np.where(np.arange(NPF)[:, None] < PTR,
                      f32(p["train_emb"])[None], f32(p["test_emb"])[None])
    shared["pf0"] = bf((f32(p["pair_pos"]) + pf_emb).T)
    shared["tf0"] = bf(f32(p["task_pos"]).T)
    shared["text0"] = bf((f32(p["task_pos"]) + f32(p["task_emb"])[None]).T)
    shared["io_emb"] = np.stack([f32(p["in_emb"]), f32(p["out_emb"])],
                                axis=1).astype(np.float32)
    shared["pair_emb4"] = np.ascontiguousarray(
        f32(p["pair_emb"])[:PP].T).astype(np.float32)
    for ln, lp in (("g", p["grid_layer"]), ("p", p["pair_layer"]),
                   ("t", p["task_layer"])):
        shared[f"{ln}_wq"] = bf(f32(lp["self"]["Wq"]) * SCALE)
        shared[f"{ln}_wk"] = bf(lp["self"]["Wk"])
        shared[f"{ln}_wv"] = bf(lp["self"]["Wv"])
        shared[f"{ln}_wo"] = bf(lp["self"]["Wo"])
        shared[f"{ln}_eq"] = bf(f32(lp["ext"]["Wq"]) * SCALE)
        shared[f"{ln}_ek"] = bf(lp["ext"]["Wk"])
        shared[f"{ln}_ev"] = bf(lp["ext"]["Wv"])
        shared[f"{ln}_eo"] = bf(lp["ext"]["Wo"])
        shared[f"{ln}_w1"] = bf(lp["W1"])
        shared[f"{ln}_w2"] = bf(lp["W2"])
    wr = np.zeros((D, D), dtype=np.float32)
    wr[:RDIM] = f32(p["grid_layer"]["self"]["Wr"])
    shared["g_wr"] = wr.astype(BF16)

    g = f32(grids).reshape(B * TWO_P, L, IN_DIM)
    pr = f32(grid_prior).reshape(B * TWO_P, L, RDIM)
    in_maps = []
    for c in range(N_CORES):
        m = dict(shared)
        xg = np.concatenate([g[2 * c], g[2 * c + 1]], axis=0).T  # [11, 1352]
        m["xg"] = np.ascontiguousarray(xg).astype(BF16)
        rel = np.zeros((D, NG), dtype=np.float32)
        rel[:RDIM] = np.concatenate([pr[2 * c], pr[2 * c + 1]], axis=0).T
        m["relx"] = rel.astype(BF16)
        in_maps.append(m)
    return in_maps


def _run_spmd(nc, in_maps, n_iters=1):
    """Execute the compiled Bass graph on 8 cores via PJRT (axon)."""
    import jax
    import jax.numpy as jnp
    from jax.sharding import Mesh, PartitionSpec
    from jax.experimental.shard_map import shard_map
    import concourse.mybir as mybir
    from concourse import bass2jax
    from concourse.bass2jax import _bass_exec_p, partition_id_tensor

    bass2jax.install_neuronx_cc_hook()

    in_names, out_names, out_avals, zero_outs = [], [], [], []
    partition_name = (nc.partition_id_tensor.name
                      if nc.partition_id_tensor else None)
    for alloc in nc.m.functions[0].allocations:
        if not isinstance(alloc, mybir.MemoryLocationSet):
            continue
        name = alloc.memorylocations[0].name
        if alloc.kind == "ExternalInput":
            if name != partition_name:
                in_names.append(name)
        elif alloc.kind == "ExternalOutput":
            shape = tuple(alloc.tensor_shape)
            dtype = mybir.dt.np(alloc.dtype)
            out_names.append(name)
            out_avals.append(jax.core.ShapedArray(shape, dtype))
            zero_outs.append(np.zeros(shape, dtype))
    n_params = len(in_names)
    n_outs = len(out_avals)
    all_in_names = list(in_names) + list(out_names)
    if partition_name is not None:
        all_in_names.append(partition_name)
    donate = tuple(range(n_params, n_params + n_outs))

    def _body(*args):
        operands = list(args)
        if partition_name is not None:
            operands.append(partition_id_tensor())
        outs = _bass_exec_p.bind(
            *operands,
            out_avals=tuple(out_avals),
            in_names=tuple(all_in_names),
            out_names=tuple(out_names),
            lowering_input_output_aliases=(),
            sim_require_finite=True,
            sim_require_nnan=True,
            nc=nc)
        return tuple(outs)

    devices = jax.devices()[:N_CORES]
    mesh = Mesh(np.asarray(devices), ("core",))
    in_specs = (PartitionSpec("core"),) * (n_params + n_outs)
    out_specs = (PartitionSpec("core"),) * n_outs
    sharded = jax.jit(
        shard_map(_body, mesh=mesh, in_specs=in_specs, out_specs=out_specs,
                  check_rep=False),
        donate_argnums=donate, keep_unused=True)

    concat_in = [
        np.concatenate([np.asarray(in_maps[c][k]) for c in range(N_CORES)],
                       axis=0)
        for k in in_names
    ]

    def one_run():
        czeros = [np.zeros((N_CORES * z.shape[0], *z.shape[1:]), z.dtype)
                  for z in zero_outs]
        outs = sharded(*concat_in, *czeros)
        jax.block_until_ready(outs)
        return outs

    out_arrs = one_run()
    times = []
    for _ in range(max(0, n_iters - 1)):
        t0 = time.perf_counter()
        out_arrs2 = one_run()
        times.append(time.perf_counter() - t0)
        del out_arrs2
    results = [
        {name: np.asarray(out_arrs[i]).reshape(N_CORES, *out_avals[i].shape)[c]
         for i, name in enumerate(out_names)}
        for c in range(N_CORES)
    ]
    return results, times


def kernel(grids, grid_prior, params, _n_timing_iters=1):
    if "nc" not in _CACHE:
        _CACHE["nc"] = _build()
    nc = _CACHE["nc"]
    in_maps = _prep_inputs(grids, grid_prior, params)
    results, times = _run_spmd(nc, in_maps, n_iters=_n_timing_iters)
    _CACHE["times"] = times
    _CACHE["results"] = results

    out = np.empty((L, B * TWO_P, OUT_DIM), dtype=np.float32)
    pf = np.empty((NPF, B * PP, D), dtype=np.float32)
    for c in range(N_CORES):
        og = results[c]["og"]            # [11, 1352]
        out[:, 2 * c, :] = og[:, :L].T
        out[:, 2 * c + 1, :] = og[:, L:].T
        pf[:, c, :] = results[c]["opf"].T
    otf = results[0]["otf"]              # [256, 256] = [D, b*128+t]
    tf = otf.reshape(D, 2, NTF).transpose(2, 1, 0).astype(np.float32)
    return out, pf, np.ascontiguousarray(tf)


# revision 54
# speedup vs baseline: 1.0921x; 1.0921x over previous
"""ArcTransformer Trainium2 kernel: 8-core SPMD, feature-major layout.

Sharding: core c owns pair (b=c//4, p=c%4) = grid columns be=2c, 2c+1.
Grid/pair stages fully local; one AllGather of pf feeds the task stage
(computed for both batches on every core; host takes core 0's copy).

Layout convention: activations are stored feature-major ("X^T"):
SBUF tile [128, 2, n] = 256 features (2 partition tiles) x n tokens.
All linear layers run with the weight as the stationary operand
(lhsT = W[d_in, d_out]); V projections run token-major (lhsT = X^T
slice) so the attention AV matmul can contract over keys.
"""

import sys
import time

sys.path.insert(0, "/opt/trn_rl_repo")

import numpy as np
import ml_dtypes

BF16 = ml_dtypes.bfloat16

# model constants (fixed by the problem)
B, TWO_P, Hh, Ww, IN_DIM = 2, 8, 26, 26, 11
D, FF, OUT_DIM, NH = 256, 1024, 11, 8
NPF, NTF, NIT, RDIM = 64, 128, 2, 155
DH = D // NH
PP = TWO_P // 2          # 4 pairs per batch
PTR = PP - 1
L = Hh * Ww              # 676
NG = 2 * L               # 1352 grid tokens per core
NEXT = NG + NTF          # 1480 pair-ext tokens
SCALE = 1.0 / np.sqrt(DH)
N_CORES = 8
EPS = 1e-5

_CACHE = {}


# ---------------------------------------------------------------- helpers
def _chunks(n, base=0, step=512):
    out = []
    c = 0
    while c < n:
        cl = min(step, n - c)
        out.append((base + c, cl))
        c += cl
    return out


def _tok_tiles(base, n, slot0):
    """[(token_offset, count, vs_slot)] in 128-token tiles."""
    out = []
    c = 0
    s = slot0
    while c < n:
        cl = min(128, n - c)
        out.append((base + c, cl, s))
        c += cl
        s += 1
    return out


# ---------------------------------------------------------------- builder
def _patch_act_tables():
    """Pin every ACT func to natural_log_exp_and_others (covers Exp/Ln/Copy/
    Relu/Square) so the greedy table assignment emits one LoadActFuncSet
    instead of thrashing between exp_and_others and natural_log (~1.3us per
    reload). Indices into act_info.json are preserved; unrelated sets are
    emptied so they can never match."""
    from concourse import bacc, hw_specs
    if getattr(bacc, "_act_tables_pinned", False):
        return
    orig = hw_specs.get_activation_tables

    def pinned(arch):
        tabs = orig(arch)
        keep = "natural_log_exp_and_others"
        if keep not in tabs:
            return tabs
        return {k: (v if k == keep else set()) for k, v in tabs.items()}

    bacc.get_activation_tables = pinned
    bacc._act_tables_pinned = True


def _build():
    import concourse.bass as bass
    import concourse.mybir as mybir
    import concourse.tile as tile
    from concourse import bacc

    _patch_act_tables()

    dt = mybir.dt
    AF = mybir.ActivationFunctionType
    OP = mybir.AluOpType
    BF = dt.bfloat16
    F32 = dt.float32

    nc = bacc.Bacc("TRN2", target_bir_lowering=False, debug=False,
                   num_devices=N_CORES)

    def din(name, shape, d=BF):
        return nc.dram_tensor(name, list(shape), d, kind="ExternalInput").ap()

    def dovt(name, shape, d=F32):
        return nc.dram_tensor(name, list(shape), d, kind="ExternalOutput").ap()

    # -------- dram inputs
    xg_d = din("xg", [IN_DIM, NG])
    relx_d = din("relx", [D, NG])
    emb_d = din("emb", [D, L])
    w_in_d = din("w_in", [IN_DIM, D])
    w_out_d = din("w_out", [D, OUT_DIM])
    pf0_d = din("pf0", [D, NPF])
    tf0_d = din("tf0", [D, NTF])
    text0_d = din("text0", [D, NTF])
    io_emb_d = din("io_emb", [D, 2], F32)
    pair_emb_d = din("pair_emb4", [D, PP], F32)
    wd_d = {}
    for ln in ("g", "p", "t"):
        for wn in ("wq", "wk", "wv", "wo", "eq", "ek", "ev", "eo"):
            wd_d[f"{ln}_{wn}"] = din(f"{ln}_{wn}", [D, D])
        wd_d[f"{ln}_w1"] = din(f"{ln}_w1", [D, FF])
        wd_d[f"{ln}_w2"] = din(f"{ln}_w2", [FF, D])
    wd_d["g_wr"] = din("g_wr", [D, D])

    og_d = dovt("og", [OUT_DIM, NG])
    opf_d = dovt("opf", [D, NPF])
    otf_d = dovt("otf", [D, 2 * NTF])

    import os
    KDBG = bool(os.environ.get("KDBG"))
    dbg_d = {}
    if KDBG:
        for nm in ("dbg_gx", "dbg_q", "dbg_kp", "dbg_vs", "dbg_e", "dbg_av",
                   "dbg_dn", "dbg_rr", "dbg_at", "dbg_y", "dbg_z"):
            dbg_d[nm] = dovt(nm, [D, 64])

    from contextlib import ExitStack
    with tile.TileContext(nc, trace_sim=False) as tc, ExitStack() as stk:
        wp = stk.enter_context(tc.tile_pool(name="wp", bufs=1))
        ap_ = stk.enter_context(tc.tile_pool(name="act", bufs=1))
        sp = stk.enter_context(tc.tile_pool(name="scr", bufs=2))
        ep = stk.enter_context(tc.tile_pool(name="expp", bufs=10))
        rp = stk.enter_context(tc.tile_pool(name="rows", bufs=2))
        pp_ = stk.enter_context(tc.tile_pool(name="ps", bufs=2, space="PSUM"))
        dp = stk.enter_context(tc.tile_pool(name="drp", bufs=1, space="DRAM"))

        # -------- load weights to SBUF  [128, K/128, M]
        # round-robin the DMA-triggering engine so the ~30 weight loads fan
        # out over four DGE queues instead of serializing on SP
        dma_engs = [nc.sync]
        wt = {}
        for wi_, (name, apd) in enumerate(wd_d.items()):
            K, M = apd.shape
            kt = max(1, K // 128)
            t = wp.tile([128, kt, M], BF, tag=f"w_{name}", name=f"w_{name}")
            dma_engs[0].dma_start(t[:],
                                        apd.rearrange("(kt p) m -> p kt m", p=128))
            wt[name] = t
        w_in_t = wp.tile([128, 1, D], BF, tag="w_w_in", name="w_w_in")
        nc.sync.dma_start(w_in_t[:IN_DIM, 0], w_in_d[:])
        w_out_t = wp.tile([128, 2, OUT_DIM], BF, tag="w_w_out", name="w_w_out")
        nc.sync.dma_start(w_out_t[:], w_out_d.rearrange("(kt p) m -> p kt m", p=128))

        ones_t = wp.tile([128, 128], BF, tag="ones_t", name="ones_t")
        nc.vector.memset(ones_t[:], 1.0)
        ones_sc = wp.tile([128, 1], BF, tag="ones_sc", name="ones_sc")
        nc.vector.memset(ones_sc[:], 1.0 / D)
        eps_t = wp.tile([128, 1], F32, tag="eps_t", name="eps_t")
        nc.vector.memset(eps_t[:], EPS)

        io_t = wp.tile([128, 2, 2], F32, tag="io_t", name="io_t")
        nc.sync.dma_start(io_t[:], io_emb_d.rearrange("(f p) n -> p f n", p=128))
        pe4_t = wp.tile([128, 2, PP], F32, tag="pe4_t", name="pe4_t")
        nc.sync.dma_start(pe4_t[:], pair_emb_d.rearrange("(f p) n -> p f n", p=128))

        dbg_t = {}
        for nm in dbg_d:
            dbg_t[nm] = wp.tile([128, 2, 64], F32, tag=nm, name=nm)

        def snap_fm(nm, t):
            if KDBG:
                for f in range(2):
                    nc.vector.tensor_copy(dbg_t[nm][:, f, :], t[:, f, 0:64])

        # -------- generic ops
        def proj(w_t, x_t, kparts, msizes, cols, evac, tag="s"):
            """out^T[m,:] = sum_k w[k,m] * x^T[k,:]; evac(ps_ap, mi, c0, cl)."""
            for mi, (moff, mcnt) in enumerate(msizes):
                for (c0, cl) in cols:
                    ps = pp_.tile([128, 1024], F32, tag=tag, name=f"ps_{tag}")
                    nk = len(kparts)
                    for ki, kc in enumerate(kparts):
                        nc.tensor.matmul(
                            ps[:mcnt, :cl],
                            lhsT=w_t[:kc, ki, moff:moff + mcnt],
                            rhs=x_t[:kc, ki, c0:c0 + cl],
                            start=(ki == 0), stop=(ki == nk - 1))
                    evac(ps, mi, c0, cl)

        def vproj(wv_t, x_t, toks, vs_t, dual=False):
            """token-major V: vs[t, kt, :] = x^T[:,t].T @ wv.
            dual: also write a copy at partitions 64.. (for dual-row attn)."""
            for (toff, tcnt, slot) in toks:
                ps = pp_.tile([128, 1024], F32, tag="s", name="ps_v")
                ps2 = pp_.tile([128, 1024], F32, tag="s", name="ps_v2") \
                    if dual else None
                for f in range(2):
                    nc.tensor.matmul(
                        ps[:tcnt, :D],
                        lhsT=x_t[:, f, toff:toff + tcnt],
                        rhs=wv_t[:, f, :],
                        start=(f == 0), stop=(f == 1))
                    if dual:
                        nc.tensor.matmul(
                            ps2[64:64 + tcnt, :D],
                            lhsT=x_t[:, f, toff:toff + tcnt],
                            rhs=wv_t[:, f, :],
                            start=(f == 0), stop=(f == 1),
                            tile_position=(0, 64))
                nc.vector.tensor_copy(vs_t[:tcnt, slot, :], ps[:tcnt, :D])
                if dual:
                    nc.vector.tensor_copy(vs_t[64:64 + tcnt, slot, :],
                                          ps2[64:64 + tcnt, :D])

        def attn_core(q_t, q_off, q_len, kp_t, vs_t, toks, out_t, dbg=False,
                      dual=False):
            """softmax(Q_h.K'_h^T).V_h for 8 heads -> out_t[:, :, q_off:+q_len].
            dual: kv <= 64 tokens; stack head pairs on partitions 0-63/64-127
            (requires vs duplicated at rows 64..)."""
            qch = _chunks(q_len)
            nt = len(toks)
            if dual:
                assert nt == 1 and toks[0][1] <= 64
                toff, tcnt, slot = toks[0]
                for f in range(2):
                    av = pp_.tile([128, 1024], F32, tag="acc", name="ps_av")
                    dn = pp_.tile([128, 1024], F32, tag="acc", name="ps_dn")
                    nc.vector.memset(av[:, :q_len], 0.0)
                    nc.vector.memset(dn[:, :q_len], 0.0)
                    for pr in range(2):
                        hb0, hb1 = 64 * pr, 64 * pr + 32
                        sps = pp_.tile([128, 1024], F32, tag="s", name="ps_sc")
                        for (c0, cl) in qch:
                            nc.tensor.matmul(
                                sps[:tcnt, c0:c0 + cl],
                                lhsT=kp_t[hb0:hb0 + 32, f, toff:toff + tcnt],
                                rhs=q_t[hb0:hb0 + 32, f,
                                        q_off + c0:q_off + c0 + cl],
                                start=True, stop=True, tile_position=(hb0, 0))
                            nc.tensor.matmul(
                                sps[64:64 + tcnt, c0:c0 + cl],
                                lhsT=kp_t[hb1:hb1 + 32, f, toff:toff + tcnt],
                                rhs=q_t[hb1:hb1 + 32, f,
                                        q_off + c0:q_off + c0 + cl],
                                start=True, stop=True, tile_position=(hb1, 64),
                                skip_group_check=True)
                        e2t = ep.tile([128, 676], BF, tag="expS", name="expS")
                        nc.scalar.activation(e2t[:, :q_len], sps[:, :q_len],
                                             AF.Exp)
                        for half, hb in ((0, hb0), (1, hb1)):
                            rbase = 64 * half
                            for (c0, cl) in qch:
                                nc.tensor.matmul(
                                    av[hb:hb + 32, c0:c0 + cl],
                                    lhsT=vs_t[rbase:rbase + tcnt, slot,
                                              f * 128 + hb:f * 128 + hb + 32],
                                    rhs=e2t[rbase:rbase + tcnt, c0:c0 + cl],
                                    start=False, stop=False,
                                    tile_position=(rbase, hb),
                                    skip_group_check=True)
                                nc.tensor.matmul(
                                    dn[hb:hb + 1, c0:c0 + cl],
                                    lhsT=ones_t[rbase:rbase + tcnt, :1],
                                    rhs=e2t[rbase:rbase + tcnt, c0:c0 + cl],
                                    start=False, stop=False,
                                    tile_position=(rbase, hb),
                                    skip_group_check=True)
                    rr = sp.tile([128, 676], BF, tag="rr", name="rr")
                    with nc.allow_low_precision(reason="softmax recip bf16"):
                        nc.vector.reciprocal(rr[:97, :q_len], dn[:97, :q_len])
                    rb = pp_.tile([128, 1024], F32, tag="acc", name="ps_rb")
                    for hh in range(4):
                        hb = 32 * hh
                        for (c0, cl) in qch:
                            nc.tensor.matmul(rb[hb:hb + 32, c0:c0 + cl],
                                             lhsT=ones_t[hb:hb + 1, :32],
                                             rhs=rr[hb:hb + 1, c0:c0 + cl],
                                             start=True, stop=True,
                                             tile_position=(hb, hb))
                    rb_sb = sp.tile([128, 676], BF, tag="rb", name="rb_sb")
                    nc.vector.tensor_copy(rb_sb[:, :q_len], rb[:, :q_len])
                    nc.vector.tensor_mul(out_t[:, f, q_off:q_off + q_len],
                                         av[:, :q_len], rb_sb[:, :q_len])
                return
            for f in range(2):
                av = pp_.tile([128, 1024], F32, tag="acc", name="ps_av")
                dn = pp_.tile([128, 1024], F32, tag="acc", name="ps_dn")
                nc.vector.memset(av[:, :q_len], 0.0)
                nc.vector.memset(dn[:, :q_len], 0.0)
                for kt, (toff, tcnt, slot) in enumerate(toks):
                    es = []
                    if False and q_len <= 128:
                        # pack the 4 heads' scores side by side -> one exp call
                        sps = pp_.tile([128, 1024], F32, tag="s", name="ps_sc")
                        for hh in range(4):
                            hb = 32 * hh
                            nc.tensor.matmul(
                                sps[:tcnt, hh * q_len:(hh + 1) * q_len],
                                lhsT=kp_t[hb:hb + 32, f, toff:toff + tcnt],
                                rhs=q_t[hb:hb + 32, f, q_off:q_off + q_len],
                                start=True, stop=True,
                                tile_position=(hb, 0))
                        e4 = ep.tile([128, 676], BF, tag="expS", name="expS")
                        nc.scalar.activation(e4[:tcnt, :4 * q_len],
                                             sps[:tcnt, :4 * q_len], AF.Exp)
                        es = [e4[:, hh * q_len:(hh + 1) * q_len]
                              for hh in range(4)]
                    else:
                        for hh in range(4):
                            hb = 32 * hh
                            sps = pp_.tile([128, 1024], F32, tag="s",
                                           name="ps_sc")
                            for (c0, cl) in qch:
                                nc.tensor.matmul(
                                    sps[:tcnt, c0:c0 + cl],
                                    lhsT=kp_t[hb:hb + 32, f, toff:toff + tcnt],
                                    rhs=q_t[hb:hb + 32, f,
                                            q_off + c0:q_off + c0 + cl],
                                    start=True, stop=True,
                                    tile_position=(hb, 0))
                            e = ep.tile([128, 676], BF, tag="expS", name="expS")
                            nc.scalar.activation(e[:tcnt, :q_len],
                                                 sps[:tcnt, :q_len], AF.Exp)
                            if dbg and f == 0 and kt == 0 and hh == 0:
                                nc.vector.tensor_copy(dbg_t["dbg_e"][:, 0, :],
                                                      e[:128, 0:64])
                            es.append(e)
                    for hh in range(4):
                        hb = 32 * hh
                        for (c0, cl) in qch:
                            nc.tensor.matmul(
                                av[hb:hb + 32, c0:c0 + cl],
                                lhsT=vs_t[:tcnt, slot, f * 128 + hb:f * 128 + hb + 32],
                                rhs=es[hh][:tcnt, c0:c0 + cl],
                                start=False, stop=False,
                                tile_position=(0, hb),
                                skip_group_check=True)
                            nc.tensor.matmul(
                                dn[hb:hb + 1, c0:c0 + cl],
                                lhsT=ones_t[:tcnt, :1],
                                rhs=es[hh][:tcnt, c0:c0 + cl],
                                start=False, stop=False,
                                tile_position=(0, hb),
                                skip_group_check=True)
                rr = sp.tile([128, 676], BF, tag="rr", name="rr")
                with nc.allow_low_precision(reason="softmax recip rows in bf16"):
                    nc.vector.reciprocal(rr[:97, :q_len], dn[:97, :q_len])
                if dbg and f == 0:
                    nc.vector.tensor_copy(dbg_t["dbg_av"][:, 0, :], av[:, 0:64])
                    nc.vector.tensor_copy(dbg_t["dbg_dn"][:, 0, :], dn[:, 0:64])
                    nc.vector.tensor_copy(dbg_t["dbg_rr"][:97, 0, :], rr[:97, 0:64])
                rb = pp_.tile([128, 1024], F32, tag="acc", name="ps_rb")
                for hh in range(4):
                    hb = 32 * hh
                    for (c0, cl) in qch:
                        nc.tensor.matmul(rb[hb:hb + 32, c0:c0 + cl],
                                         lhsT=ones_t[hb:hb + 1, :32],
                                         rhs=rr[hb:hb + 1, c0:c0 + cl],
                                         start=True, stop=True,
                                         tile_position=(hb, hb))
                rb_sb = sp.tile([128, 676], BF, tag="rb", name="rb_sb")
                nc.vector.tensor_copy(rb_sb[:, :q_len], rb[:, :q_len])
                nc.vector.tensor_mul(out_t[:, f, q_off:q_off + q_len],
                                     av[:, :q_len], rb_sb[:, :q_len])

        def layer_norm(y_t, n, z_t):
            """z = (y - mu)/sqrt(var+eps), per token (feature-dim stats)."""
            sq = sp.tile([128, 2, NEXT], BF, tag="sq", name="sq", bufs=1)
            for f in range(2):
                nc.vector.tensor_mul(sq[:, f, :n], y_t[:, f, :n], y_t[:, f, :n])
            mu = rp.tile([1, NEXT], F32, tag="mu", name="mu", bufs=1)
            e2 = rp.tile([1, NEXT], F32, tag="e2", name="e2", bufs=1)
            for (c0, cl) in _chunks(n):
                for src, dst in ((y_t, mu), (sq, e2)):
                    ps = pp_.tile([128, 1024], F32, tag="s", name="ps_st")
                    for f in range(2):
                        nc.tensor.matmul(ps[:1, :cl],
                                         lhsT=ones_sc[:, :1],
                                         rhs=src[:, f, c0:c0 + cl],
                                         start=(f == 0), stop=(f == 1))
                    nc.vector.tensor_copy(dst[:1, c0:c0 + cl], ps[:1, :cl])
            rs1 = rp.tile([1, NEXT], F32, tag="rs1", name="rs1", bufs=1)
            rs2 = rp.tile([1, NEXT], F32, tag="rs2", name="rs2", bufs=1)
            nc.vector.tensor_mul(rs1[:1, :n], mu[:1, :n], mu[:1, :n])    # mu^2
            nc.vector.tensor_sub(rs2[:1, :n], e2[:1, :n], rs1[:1, :n])   # var
            nc.scalar.activation(rs1[:1, :n], rs2[:1, :n], AF.Ln,
                                 bias=eps_t[:1, :1])
            a_r = rs2
            nc.scalar.activation(a_r[:1, :n], rs1[:1, :n], AF.Exp, scale=-0.5)
            c_r = rs1
            nc.vector.scalar_tensor_tensor(c_r[:1, :n], mu[:1, :n], -1.0,
                                           a_r[:1, :n], OP.mult, OP.mult)
            ab = sp.tile([128, NEXT], F32, tag="ab", name="ab", bufs=1)
            cb = sp.tile([128, NEXT], F32, tag="cb", name="cb", bufs=1)
            nc.gpsimd.partition_broadcast(ab[:, :n], a_r[:1, :n])
            nc.gpsimd.partition_broadcast(cb[:, :n], c_r[:1, :n])
            for f in range(2):
                nc.vector.tensor_mul(z_t[:, f, :n], y_t[:, f, :n], ab[:, :n])
                nc.vector.tensor_add(z_t[:, f, :n], z_t[:, f, :n], cb[:, :n])

        D2 = [128, 128]
        M2 = [(0, 128), (128, 128)]

        def xf_layer(pre, x_t, n, win_self, vs_self_n, ext_t, ext_n, win_ext,
                     vs_ext_n, out_t, krel_t=None, dbg=False):
            """one transformer layer; LN3 output written to out_t[:, :, :n]."""
            cols = _chunks(n)
            q_t = ap_.tile([128, 2, NG], BF, tag="q_t", name="q_t")
            kp_t = ap_.tile([128, 2, NEXT], BF, tag="kp_t", name="kp_t")
            at_t = ap_.tile([128, 2, NG], BF, tag="at_t", name="at_t")
            y_t = ap_.tile([128, 2, NG], BF, tag="y_t", name="y_t")
            z1_t = ap_.tile([128, 2, NG], BF, tag="z1_t", name="z1_t")
            z2_t = ap_.tile([128, 2, NG], BF, tag="z2_t", name="z2_t")
            vs_t = ap_.tile([128, 12, D], BF, tag="vs_t", name="vs_t")

            def cp(dst):
                def f(ps, mi, c0, cl):
                    nc.any.tensor_copy(dst[:, mi, c0:c0 + cl], ps[:, :cl])
                return f

            def addk(dst, k_t):
                def f(ps, mi, c0, cl):
                    nc.vector.tensor_add(dst[:, mi, c0:c0 + cl], ps[:, :cl],
                                         k_t[:, mi, c0:c0 + cl])
                return f

            # ---- self attention
            proj(wt[pre + "_wq"], x_t, D2, M2, cols, cp(q_t))
            if krel_t is not None:
                proj(wt[pre + "_wk"], x_t, D2, M2, cols, addk(kp_t, krel_t))
            else:
                proj(wt[pre + "_wk"], x_t, D2, M2, cols, cp(kp_t))
            alltoks = [t for (_, _, tl) in win_self for t in tl]
            vproj(wt[pre + "_wv"], x_t, alltoks, vs_t)
            if dbg:
                snap_fm("dbg_q", q_t)
                snap_fm("dbg_kp", kp_t)
                nc.vector.tensor_copy(dbg_t["dbg_vs"][:, 0, :],
                                      vs_t[:128, 0, 0:64])
                nc.vector.tensor_copy(dbg_t["dbg_vs"][:, 1, :],
                                      vs_t[:128, 0, 64:128])
            for wi, (qo, ql, tl) in enumerate(win_self):
                attn_core(q_t, qo, ql, kp_t, vs_t, tl, at_t,
                          dbg=(dbg and wi == 0))
            if dbg:
                snap_fm("dbg_at", at_t)
            proj(wt[pre + "_wo"], at_t, D2, M2, cols, addk(y_t, x_t))
            layer_norm(y_t, n, z1_t)
            if dbg:
                snap_fm("dbg_y", y_t)
                snap_fm("dbg_z", z1_t)
            # ---- external attention (K2/V2 only depend on ext_t, so they can
            # overlap the self-attention + LN1 work -> separate tiles)
            kp2_t = ap_.tile([128, 2, NEXT], BF, tag="kp2_t", name="kp2_t")
            proj(wt[pre + "_eq"], z1_t, D2, M2, cols, cp(q_t))
            proj(wt[pre + "_ek"], ext_t, D2, M2, _chunks(ext_n), cp(kp2_t))
            exttoks = [t for (_, _, tl) in win_ext for t in tl]
            seen = set()
            uniq = [t for t in exttoks if not (t in seen or seen.add(t))]
            ext_dual = False
            vproj(wt[pre + "_ev"], ext_t, uniq, vs_t, dual=ext_dual)
            for (qo, ql, tl) in win_ext:
                attn_core(q_t, qo, ql, kp2_t, vs_t, tl, at_t, dual=ext_dual)
            proj(wt[pre + "_eo"], at_t, D2, M2, cols, addk(y_t, z1_t))
            layer_norm(y_t, n, z2_t)
            # ---- FFN
            h_t = ap_.tile([128, 8, NG], BF, tag="h_t", name="h_t")

            def relu_evac(ps, mi, c0, cl):
                if mi % 2 == 0:
                    nc.scalar.activation(h_t[:, mi, c0:c0 + cl], ps[:, :cl],
                                         AF.Relu)
                else:
                    nc.vector.tensor_scalar_max(h_t[:, mi, c0:c0 + cl],
                                                ps[:, :cl], 0.0)

            proj(wt[pre + "_w1"], z2_t, D2, [(i * 128, 128) for i in range(8)],
                 cols, relu_evac)
            proj(wt[pre + "_w2"], h_t, [128] * 8, M2, cols, addk(y_t, z2_t))
            layer_norm(y_t, n, out_t)

        # ============================ forward ============================
        # ---- grid embedding
        gx = ap_.tile([128, 2, NEXT], BF, tag="gx", name="gx")
        xg_t = ap_.tile([128, 1, NG], BF, tag="q_t", name="xg_t")
        nc.sync.dma_start(xg_t[:IN_DIM, 0], xg_d[:])
        emb_t = ap_.tile([128, 2, L], BF, tag="kp_t", name="emb_t")
        nc.sync.dma_start(emb_t[:], emb_d.rearrange("(f p) n -> p f n", p=128))
        nc.sync.dma_start(gx[:, :, NG:NEXT],
                          text0_d.rearrange("(f p) n -> p f n", p=128))

        def emb_evac(ps, mi, c0, cl):
            ec = c0 % L
            nc.vector.tensor_add(gx[:, mi, c0:c0 + cl], ps[:, :cl],
                                 emb_t[:, mi, ec:ec + cl])

        gcols = _chunks(L) + _chunks(L, base=L)
        proj(w_in_t, xg_t, [IN_DIM], M2, gcols, emb_evac)
        snap_fm("dbg_gx", gx)

        # ---- cached rel-prior projection (added to K in both grid layers)
        relx_t = ap_.tile([128, 2, NG], BF, tag="at_t", name="relx_t")
        nc.sync.dma_start(relx_t[:], relx_d.rearrange("(f p) n -> p f n", p=128))
        krel_t = ap_.tile([128, 2, NG], BF, tag="krel_t", name="krel_t")

        def krel_evac(ps, mi, c0, cl):
            nc.any.tensor_copy(krel_t[:, mi, c0:c0 + cl], ps[:, :cl])

        proj(wt["g_wr"], relx_t, D2, M2, _chunks(NG), krel_evac)

        # ---- pf0 / tf0
        pf_t = ap_.tile([128, 2, NPF], BF, tag="pf_t", name="pf_t")
        nc.sync.dma_start(pf_t[:], pf0_d.rearrange("(f p) n -> p f n", p=128))

        g_self_win = [(0, L, _tok_tiles(0, L, 0)), (L, L, _tok_tiles(L, L, 6))]
        g_ext_win = [(0, L, [(0, NPF, 0)]), (L, L, [(0, NPF, 0)])]

        # ---- grid layer, iteration 0 (ext kv = pf0)
        xf_layer("g", gx, NG, g_self_win, 12, pf_t, NPF, g_ext_win, 1,
                 gx, krel_t=krel_t, dbg=KDBG)

        # ---- in/out embeddings persist into g
        for f in range(2):
            for io in range(2):
                nc.vector.tensor_scalar_add(gx[:, f, io * L:(io + 1) * L],
                                            gx[:, f, io * L:(io + 1) * L],
                                            io_t[:, f, io:io + 1])

        # ---- pair layer: target pf0 -> pf1; ext = [g4 | text0] = gx
        pf1_t = ap_.tile([128, 2, NPF], BF, tag="pf1_t", name="pf1_t")
        p_self_win = [(0, NPF, [(0, NPF, 0)])]
        p_ext_win = [(0, NPF, _tok_tiles(0, NEXT, 0))]
        xf_layer("p", pf_t, NPF, p_self_win, 1, gx, NEXT, p_ext_win, 12, pf1_t)

        # pf output (f32) + collective input bounce
        opf_sb = ap_.tile([128, 2, NPF], F32, tag="opf_sb", name="opf_sb")
        for f in range(2):
            nc.any.tensor_copy(opf_sb[:, f, :], pf1_t[:, f, :])
        nc.sync.dma_start(opf_d.rearrange("(f p) n -> p f n", p=128), opf_sb[:])
        pf_bnc = dp.tile([D, NPF], BF, tag="pf_bnc", name="pf_bnc")
        nc.gpsimd.dma_start(pf_bnc.rearrange("(f p) n -> p f n", p=128), pf1_t[:])
        pf_gth = dp.tile([N_CORES, D, NPF], BF, tag="pf_gth", name="pf_gth",
                         addr_space="Shared")
        nc.gpsimd.collective_compute(
            "AllGather", mybir.AluOpType.bypass,
            replica_groups=[list(range(N_CORES))],
            ins=[pf_bnc[:].opt()], outs=[pf_gth[:].opt()])

        # ---- grid layer, iteration 1 (ext kv = own pf1 column)
        xf_layer("g", gx, NG, g_self_win, 12, pf1_t, NPF, g_ext_win, 1,
                 gx, krel_t=krel_t)

        # ---- grid output projection
        og_sb = ap_.tile([128, NG], F32, tag="og_sb", name="og_sb")

        def og_evac(ps, mi, c0, cl):
            nc.any.tensor_copy(og_sb[:OUT_DIM, c0:c0 + cl], ps[:OUT_DIM, :cl])

        proj(w_out_t, gx, D2, [(0, OUT_DIM)], _chunks(NG), og_evac)
        nc.sync.dma_start(og_d[:], og_sb[:OUT_DIM, :])

        # ---- task layer inputs: ext2 from gathered pf (both batches)
        ext2_t = ap_.tile([128, 2, 512], BF, tag="ext2_t", name="ext2_t")
        g2 = pf_gth.rearrange("lo (f p) (p4 bb hi) -> f bb p p4 hi lo",
                              p=128, bb=2, hi=8)
        for f in range(2):
            for b in range(2):
                dst = ext2_t[:, f, b * 256:(b + 1) * 256].rearrange(
                    "p (p4 hi lo) -> p p4 hi lo", p4=4, hi=8)
                for lo in range(8):
                    nc.sync.dma_start(dst[:, :, :, lo], g2[f, b, :, :, :, lo])
        for f in range(2):
            for b in range(2):
                for p4 in range(PP):
                    c0 = b * 256 + p4 * 64
                    nc.vector.tensor_scalar_add(ext2_t[:, f, c0:c0 + 64],
                                                ext2_t[:, f, c0:c0 + 64],
                                                pe4_t[:, f, p4:p4 + 1])

        tf_t = ap_.tile([128, 2, 2 * NTF], BF, tag="tf_t", name="tf_t")
        for b in range(2):
            nc.sync.dma_start(tf_t[:, :, b * NTF:(b + 1) * NTF],
                              tf0_d.rearrange("(f p) n -> p f n", p=128))
        tf1_t = ap_.tile([128, 2, 2 * NTF], BF, tag="tf1_t", name="tf1_t")
        t_self_win = [(0, NTF, [(0, NTF, 0)]), (NTF, NTF, [(NTF, NTF, 1)])]
        t_ext_win = [(0, NTF, [(0, 128, 0), (128, 128, 1)]),
                     (NTF, NTF, [(256, 128, 2), (384, 128, 3)])]
        xf_layer("t", tf_t, 2 * NTF, t_self_win, 2, ext2_t, 512, t_ext_win, 4,
                 tf1_t)
        otf_sb = ap_.tile([128, 2, 2 * NTF], F32, tag="otf_sb", name="otf_sb")
        for f in range(2):
            nc.any.tensor_copy(otf_sb[:, f, :], tf1_t[:, f, :])
        nc.sync.dma_start(otf_d.rearrange("(f p) n -> p f n", p=128), otf_sb[:])

        for nm in dbg_d:
            nc.sync.dma_start(dbg_d[nm].rearrange("(f p) n -> p f n", p=128),
                              dbg_t[nm][:])

    import os
    if not os.environ.get("KERNEL_BUILD_ONLY"):
        nc.compile()
    return nc


# ---------------------------------------------------------------- host side
def _np32(x):
    return np.asarray(x, dtype=np.float32)


def _prep_inputs(grids, grid_prior, params):
    p = params
    f32 = _np32

    def bf(x):
        return np.ascontiguousarray(f32(x)).astype(BF16)

    shared = {}
    # grid embedding (pos added twice + seq embeddings + b_in), transposed
    gpos = f32(p["grid_pos"])[:Ww, :Hh, :].reshape(L, D)
    seq = np.where(np.arange(L)[:, None] < PTR * 2,
                   f32(p["train_emb"])[None], f32(p["test_emb"])[None])
    seq[L - 1] += f32(p["test_out_emb"])
    emb = f32(p["b_in"])[None] + 2.0 * gpos + seq
    shared["emb"] = bf(emb.T)
    shared["w_in"] = bf(p["W_in"])
    shared["w_out"] = bf(p["W_out"])
    pf_emb = np.where(np.arange(NPF)[:, None] < PTR,
                      f32(p["train_emb"])[None], f32(p["test_emb"])[None])
    shared["pf0"] = bf((f32(p["pair_pos"]) + pf_emb).T)
    shared["tf0"] = bf(f32(p["task_pos"]).T)
    shared["text0"] = bf((f32(p["task_pos"]) + f32(p["task_emb"])[None]).T)
    shared["io_emb"] = np.stack([f32(p["in_emb"]), f32(p["out_emb"])],
                                axis=1).astype(np.float32)
    shared["pair_emb4"] = np.ascontiguousarray(
        f32(p["pair_emb"])[:PP].T).astype(np.float32)
    for ln, lp in (("g", p["grid_layer"]), ("p", p["pair_layer"]),
                   ("t", p["task_layer"])):
        shared[f"{ln}_wq"] = bf(f32(lp["self"]["Wq"]) * SCALE)
        shared[f"{ln}_wk"] = bf(lp["self"]["Wk"])
        shared[f"{ln}_wv"] = bf(lp["self"]["Wv"])
        shared[f"{ln}_wo"] = bf(lp["self"]["Wo"])
        shared[f"{ln}_eq"] = bf(f32(lp["ext"]["Wq"]) * SCALE)
        shared[f"{ln}_ek"] = bf(lp["ext"]["Wk"])
        shared[f"{ln}_ev"] = bf(lp["ext"]["Wv"])
        shared[f"{ln}_eo"] = bf(lp["ext"]["Wo"])
        shared[f"{ln}_w1"] = bf(lp["W1"])
        shared[f"{ln}_w2"] = bf(lp["W2"])
    wr = np.zeros((D, D), dtype=np.float32)
    wr[:RDIM] = f32(p["grid_layer"]["self"]["Wr"])
    shared["g_wr"] = wr.astype(BF16)

    g = f32(grids).reshape(B * TWO_P, L, IN_DIM)
    pr = f32(grid_prior).reshape(B * TWO_P, L, RDIM)
    in_maps = []
    for c in range(N_CORES):
        m = dict(shared)
        xg = np.concatenate([g[2 * c], g[2 * c + 1]], axis=0).T  # [11, 1352]
        m["xg"] = np.ascontiguousarray(xg).astype(BF16)
        rel = np.zeros((D, NG), dtype=np.float32)
        rel[:RDIM] = np.concatenate([pr[2 * c], pr[2 * c + 1]], axis=0).T
        m["relx"] = rel.astype(BF16)
        in_maps.append(m)
    return in_maps


def _run_spmd(nc, in_maps, n_iters=1):
    """Execute the compiled Bass graph on 8 cores via PJRT (axon)."""
    import jax
    import jax.numpy as jnp
    from jax.sharding import Mesh, PartitionSpec
    from jax.experimental.shard_map import shard_map
    import concourse.mybir as mybir
    from concourse import bass2jax
    from concourse.bass2jax import _bass_exec_p, partition_id_tensor

    bass2jax.install_neuronx_cc_hook()

    in_names, out_names, out_avals, zero_outs = [], [], [], []
    partition_name = (nc.partition_id_tensor.name
                      if nc.partition_id_tensor else None)
    for alloc in nc.m.functions[0].allocations:
        if not isinstance(alloc, mybir.MemoryLocationSet):
            continue
        name = alloc.memorylocations[0].name
        if alloc.kind == "ExternalInput":
            if name != partition_name:
                in_names.append(name)
        elif alloc.kind == "ExternalOutput":
            shape = tuple(alloc.tensor_shape)
            dtype = mybir.dt.np(alloc.dtype)
            out_names.append(name)
            out_avals.append(jax.core.ShapedArray(shape, dtype))
            zero_outs.append(np.zeros(shape, dtype))
    n_params = len(in_names)
    n_outs = len(out_avals)
    all_in_names = list(in_names) + list(out_names)
    if partition_name is not None:
        all_in_names.append(partition_name)
    donate = tuple(range(n_params, n_params + n_outs))

    def _body(*args):
        operands = list(args)
        if partition_name is not None:
            operands.append(partition_id_tensor())
        outs = _bass_exec_p.bind(
            *operands,
            out_avals=tuple(out_avals),
            in_names=tuple(all_in_names),
            out_names=tuple(out_names),
            lowering_input_output_aliases=(),
            sim_require_finite=True,
            sim_require_nnan=True,
            nc=nc)
        return tuple(outs)

    devices = jax.devices()[:N_CORES]
    mesh = Mesh(np.asarray(devices), ("core",))
    in_specs = (PartitionSpec("core"),) * (n_params + n_outs)
    out_specs = (PartitionSpec("core"),) * n_outs
    sharded = jax.jit(
        shard_map(_body, mesh=mesh, in_specs=in_specs, out_specs=out_specs,
                  check_rep=False),
        donate_argnums=donate, keep_unused=True)

    concat_in = [
        np.concatenate([np.asarray(in_maps[c][k]) for c in range(N_CORES)],
                       axis=0)
        for k in in_names
    ]

    def one_run():
        czeros = [np.zeros((N_CORES * z.shape[0], *z.shape[1:]), z.dtype)
                  for z in zero_outs]
        outs = sharded(*concat_in, *czeros)
        jax.block_until_ready(outs)
        return outs

    out_arrs = one_run()
    times = []
    for _ in range(max(0, n_iters - 1)):
        t0 = time.perf_counter()
        out_arrs2 = one_run()
        times.append(time.perf_counter() - t0)
        del out_arrs2
    results = [
        {name: np.asarray(out_arrs[i]).reshape(N_CORES, *out_avals[i].shape)[c]
         for i, name in enumerate(out_names)}
        for c in range(N_CORES)
    ]
    return results, times


def kernel(grids, grid_prior, params, _n_timing_iters=1):
    if "nc" not in _CACHE:
        _CACHE["nc"] = _build()
    nc = _CACHE["nc"]
    in_maps = _prep_inputs(grids, grid_prior, params)
    results, times = _run_spmd(nc, in_maps, n_iters=_n_timing_iters)
    _CACHE["times"] = times
    _CACHE["results"] = results

    out = np.empty((L, B * TWO_P, OUT_DIM), dtype=np.float32)
    pf = np.empty((NPF, B * PP, D), dtype=np.float32)
    for c in range(N_CORES):
        og = results[c]["og"]            # [11, 1352]
        out[:, 2 * c, :] = og[:, :L].T
        out[:, 2 * c + 1, :] = og[:, L:].T
        pf[:, c, :] = results[c]["opf"].T
    otf = results[0]["otf"]              # [256, 256] = [D, b*128+t]
    tf = otf.reshape(D, 2, NTF).transpose(2, 1, 0).astype(np.float32)
    return out, pf, np.ascontiguousarray(tf)
